# revision 26
# baseline (speedup 1.0000x reference)
"""Trainium2 Bass kernel for nn_DependencyEncoder3 (tree-LSTM dependency encoder).

Model: perfect 8-ary tree, 6 levels, level-order indexing, D=512, P=8 dep types.
  leaves: z = final cell state of a 1-step LSTM over [x]  (c = sig(i)*tanh(g))
  level l: per node, children codes are dep-transformed (z_c -> W_dep[p] z_c +
  b_dep[p]) and fed as a 9-step sequence [zc_0..zc_7, x_own] through the LSTM;
  z = final c.  Output: root z, [1, 512].

Sharding: core c owns the entire subtree of level-1 node c.  Levels 5..1 are
fully core-local; the only communication is a 16 KB AllGather of the eight
level-1 codes before the (replicated) root computation.

Layout notes (all engines balanced under a saturated PE):
 - activations evacuate PSUM with the gate bias fused (Act engine), so the
   vector engine only does the c/h elementwise chain.
 - L4 recurrence: full-width (512) matmuls into 4-gate PSUM tiles whose rows
   are whole PSUM banks (so ih/hh accumulation groups may interleave without
   the has_written bank-clear hazard); the i/f tiles' W_ih matmuls form an
   h-independent prefix that covers the h(t-1) chain latency.
 - all child gathers run as GPSIMD ap_gather on child-major [128, n, 4]
   layouts (~6x faster than per-chunk indirect_copy), issued steps ahead of
   the recurrence and repacked to chunk-major by the vector engine off the
   critical path.
 - small levels preload the batched input projection (gin) into PSUM with an
   identity matmul so W_hh accumulates on top, keeping the 9-step serial
   chain short; tiny keep-warm matmuls punctuate the chain so the PE clock
   gate stays at full rate.
"""

import sys

for _p in ("/opt/trn_rl_repo", "/root/.axon_site/_ro/trn_rl_repo"):
    if _p not in sys.path:
        sys.path.append(_p)

import numpy as np
import ml_dtypes
from contextlib import ExitStack

import concourse.bass as bass
import concourse.tile as tile
from concourse import bacc, mybir
from concourse.bass_utils import run_bass_kernel_spmd

F32 = mybir.dt.float32
BF16 = mybir.dt.bfloat16
U16 = mybir.dt.uint16
I16 = mybir.dt.int16
AFT = mybir.ActivationFunctionType
BF = ml_dtypes.bfloat16

NCORES = 8
D = 512
KC = 4            # feature chunks of 128
G = 2048          # gate width
P = 8             # dep types
K = 8             # children per node
OFF = [0, 1, 9, 73, 585, 4681, 37449]
LEAF = 4096       # leaves per core

# (name, C=children per core, m=nodes per core, xoff into xown cols, is_root)
LEVEL_SHAPES = [
    ("L4", 4096, 512, 0, False),
    ("L3", 512, 64, 512, False),
    ("L2", 64, 8, 576, False),
    ("L1", 8, 1, 584, False),
    ("RT", 8, 1, 585, True),
]
XOWN_COLS = 586


def _ceil16(x):
    return -(-x // 16) * 16


def _ceil32(x):
    # indirect_copy idx slices must be 4B-aligned: widths in 32-index units
    return -(-x // 32) * 32


def _wrap_idx(vals, ncols):
    """Wrap indices into the GPSIMD per-16-partition layout, replicated to 128
    partitions: idx[p, s] = vals[s*16 + (p % 16)]."""
    arr = np.zeros((16, ncols), dtype=np.uint16)
    v = np.asarray(vals, dtype=np.uint16)
    n = len(v)
    pad = np.zeros(ncols * 16, dtype=np.uint16)
    pad[:n] = v
    arr[:, :] = pad.reshape(ncols, 16).T
    return np.tile(arr, (8, 1))


def _chunks(total, step):
    out = []
    o = 0
    while o < total:
        out.append((o, min(step, total - o)))
        o += step
    return out


def build_program(caps, n_levels=len(LEVEL_SHAPES), n_iters=1,
                  stop_after=None, unroll=False):
    """Build the uniform SPMD program.  caps: per-level dep-group capacity.
    n_levels < 5 truncates the level sweep (timing experiments only).
    n_iters > 1 repeats the body, via an in-NEFF For_i loop or (unroll=True,
    collective-safe) python unrolling (timing only)."""
    nc = bacc.Bacc("TRN2", debug=False, num_devices=NCORES)

    leaf_pad = P * caps[0]
    xleaf_p = nc.declare_dram_parameter("xleaf", [D, leaf_pad], BF16, isOutput=False)
    xown_p = nc.declare_dram_parameter("xown", [D, XOWN_COLS], BF16, isOutput=False)
    wih_p = nc.declare_dram_parameter("wih_t", [D, G], BF16, isOutput=False)
    whh_p = nc.declare_dram_parameter("whh_t", [D, G], BF16, isOutput=False)
    wdep_p = nc.declare_dram_parameter("wdep_t", [P, D, D], BF16, isOutput=False)
    bsum_p = nc.declare_dram_parameter("bsum_t", [128, 16], F32, isOutput=False)
    ident_p = nc.declare_dram_parameter("ident", [128, 128], BF16, isOutput=False)
    bdep_p = nc.declare_dram_parameter("bdep_t", [128, 32], F32, isOutput=False)

    giw = [0 if i == 0 else P * caps[i] // 16
           for i in range(len(LEVEL_SHAPES))]
    siw = [K * _ceil32(m) // 16 for (_, _, m, _, _) in LEVEL_SHAPES]
    idxg_p = nc.declare_dram_parameter("idxg", [128, sum(giw)], U16, isOutput=False)
    idxs_p = nc.declare_dram_parameter("idxs", [128, sum(siw)], U16, isOutput=False)
    out_p = nc.declare_dram_parameter("out", [1, D], F32, isOutput=True)

    with ExitStack() as outer:
        tc = outer.enter_context(tile.TileContext(nc))
        if n_iters > 1 and not unroll:
            outer.enter_context(tc.For_i(0, n_iters))
        for _rep in range(n_iters if unroll else 1):
            _body(nc, tc, caps, n_levels, stop_after, locals())
    nc.finalize()
    return nc


def _body(nc, tc, caps, n_levels, stop_after, env):
    leaf_pad = env["leaf_pad"]
    xleaf_p = env["xleaf_p"]; xown_p = env["xown_p"]; wih_p = env["wih_p"]
    whh_p = env["whh_p"]; wdep_p = env["wdep_p"]; bsum_p = env["bsum_p"]
    bdep_p = env["bdep_p"]; ident_p = env["ident_p"]; giw = env["giw"]
    siw = env["siw"]; idxg_p = env["idxg_p"]; idxs_p = env["idxs_p"]
    out_p = env["out_p"]
    with ExitStack() as ctx:
        wpool = ctx.enter_context(tc.tile_pool(name="w", bufs=1))
        zpool = ctx.enter_context(tc.tile_pool(name="z", bufs=1))
        gpool = ctx.enter_context(tc.tile_pool(name="g", bufs=2))
        npool = ctx.enter_context(tc.tile_pool(name="nl", bufs=2))
        cmpool = ctx.enter_context(tc.tile_pool(name="cm", bufs=1))
        dram = ctx.enter_context(tc.tile_pool(name="dram", bufs=1, space="DRAM"))

        # ---- persistent weights / indices ----
        wih = wpool.tile([128, KC, G], BF16)
        nc.sync.dma_start(out=wih, in_=wih_p.ap().rearrange("(c p) g -> p c g", p=128))
        bsum = wpool.tile([128, 16], F32)
        nc.sync.dma_start(out=bsum, in_=bsum_p.ap())
        bdep = wpool.tile([128, 32], F32)
        nc.sync.dma_start(out=bdep, in_=bdep_p.ap())
        ident = wpool.tile([128, 128], BF16)
        nc.sync.dma_start(out=ident, in_=ident_p.ap())

        # ---- leaf stage: z5 = sigmoid(i) * tanh(g), gates from x @ W_ih.T ----
        # PSUM: separate i / g tiles (8 KB each) sharing 2 slots, so block
        # b+1's matmuls overlap block b's activations.
        zcs0_stack = ExitStack()
        zcs0_pool = zcs0_stack.enter_context(tc.tile_pool(name="zcs0", bufs=1))
        pz5_stack = ExitStack()
        pz5 = pz5_stack.enter_context(tc.tile_pool(name="pz5", bufs=1))
        z5 = pz5.tile([128, KC, leaf_pad], BF16, tag="z5", name="z5")
        with tc.tile_pool(name="psl", bufs=2, space="PSUM") as psl:
            for nb in range(leaf_pad // 512):
                sl = slice(nb * 512, (nb + 1) * 512)
                xt = gpool.tile([128, KC, 512], BF16, tag="zct")
                nc.sync.dma_start(
                    out=xt,
                    in_=xleaf_p.ap().rearrange("(c p) n -> p c n", p=128)[:, :, sl],
                )
                pi = psl.tile([128, KC, 512], F32, tag="pl")
                for r in range(KC):
                    for kb in range(KC):
                        nc.tensor.matmul(
                            pi[:, r, :], wih[:, kb, r * 128:(r + 1) * 128],
                            xt[:, kb, :],
                            start=(kb == 0), stop=(kb == KC - 1),
                        )
                pg = psl.tile([128, KC, 512], F32, tag="pl")
                for r in range(KC):
                    mmg = 12 + r
                    for kb in range(KC):
                        nc.tensor.matmul(
                            pg[:, r, :], wih[:, kb, mmg * 128:(mmg + 1) * 128],
                            xt[:, kb, :],
                            start=(kb == 0), stop=(kb == KC - 1),
                        )
                si = gpool.tile([128, KC, 512], BF16, tag="lnl")
                for r in range(KC):
                    nc.scalar.activation(out=si[:, r, :], in_=pi[:, r, :],
                                         func=AFT.Sigmoid,
                                         bias=bsum[:, r:r + 1])
                tg = gpool.tile([128, KC, 512], BF16, tag="lnl")
                for r in range(KC):
                    nc.scalar.activation(out=tg[:, r, :], in_=pg[:, r, :],
                                         func=AFT.Tanh,
                                         bias=bsum[:, 12 + r:13 + r])
                nc.vector.tensor_mul(z5[:, :, sl], si, tg)

        # ---- remaining persistent loads (issued after the leaf stage so the
        # leaf's xleaf traffic goes first; these drain during leaf compute) ----
        whh = wpool.tile([128, KC, G], BF16)
        nc.sync.dma_start(out=whh, in_=whh_p.ap().rearrange("(c p) g -> p c g", p=128))
        xown = wpool.tile([128, KC, XOWN_COLS], BF16)
        nc.sync.dma_start(
            out=xown, in_=xown_p.ap().rearrange("(c p) n -> p c n", p=128)
        )
        idxg = wpool.tile([128, sum(giw)], U16)
        nc.sync.dma_start(out=idxg, in_=idxg_p.ap())
        idxs = wpool.tile([128, sum(siw)], U16)
        nc.sync.dma_start(out=idxs, in_=idxs_p.ap())

        # ---- level sweep ----
        env_small = {}
        h = zpool.tile([128, KC, 512], BF16, tag="h", name="h")
        c = zpool.tile([128, KC, 512], F32, tag="c", name="c")
        goff = 0
        soff = 0
        zin = z5
        c_final = None
        for li, (lname, C, m, xoff, is_root) in enumerate(
                LEVEL_SHAPES[:max(n_levels, 0)]):
            cap = caps[li]
            Cs = P * cap
            mw = _ceil32(m)
            big = m >= 256

            if is_root:
                # AllGather the eight level-1 codes; every core computes the
                # true root (identical inputs everywhere).
                cc_in = dram.tile([1, D], BF16)
                nc.gpsimd.dma_start(
                    out=cc_in.rearrange("a (c p) -> p c a", p=128),
                    in_=zin[:, 0:1, :].rearrange("p n c -> p c n"),
                )
                cc_out = dram.tile([NCORES, D], BF16)
                nc.gpsimd.collective_compute(
                    "AllGather",
                    mybir.AluOpType.bypass,
                    replica_groups=[list(range(NCORES))],
                    ins=[cc_in.opt()],
                    outs=[cc_out.opt()],
                )
                zrt = zpool.tile([128, K, KC], BF16, tag="zrt")
                nc.gpsimd.dma_start(
                    out=zrt,
                    in_=cc_out.rearrange("n (c p) -> p n c", p=128),
                )
                zin = zrt

            if li == 2:
                # L2/L1/root re-use one resident copy of all dep weights
                # instead of re-streaming 4 MB per level; opened before this
                # level's pools so pool open/close stays LIFO.
                env_small["pool"] = ExitStack()
                wdpool = env_small["pool"].enter_context(
                    tc.tile_pool(name="wdall", bufs=1))
                env_small["wdall"] = wdpool.tile(
                    [128, KC, P, D], BF16, name="wdall")
                for pq in range(P):
                    nc.sync.dma_start(
                        out=env_small["wdall"][:, :, pq, :],
                        in_=wdep_p.ap().rearrange(
                            "d (c p) m -> d p c m", p=128)[pq],
                    )

            # --- dep transform: zcs[:, :, p*cap + j] = Wdep[p] @ z_sorted + bdep[p]
            lvl_stack = ExitStack()
            pgin = (None if big else
                    lvl_stack.enter_context(
                        tc.tile_pool(name=f"pgin{li}", bufs=1)))
            # zcs is child-major [128, child, KC] so ap_gather (vectorized
            # GPSIMD gather, ~6x indirect_copy) can fetch whole children.
            if li == 0:
                zcs_pool = zcs0_pool
            else:
                zcs_pool = lvl_stack.enter_context(
                    tc.tile_pool(name=f"zcs{li}", bufs=1))
            zcs = zcs_pool.tile([128, max(Cs, 128), KC], BF16, tag="zcs",
                                name=f"zcs{li}")
            with tc.tile_pool(name=f"psd{li}", bufs=6, space="PSUM") as psd:
                small_resident = li >= 2
                for p in range(P):
                    if li == 0:
                        # leaves arrive dep-sorted from the host: groups are
                        # plain column ranges of z5, no gather needed.
                        zs = zin[:, :, p * cap:(p + 1) * cap]
                    elif p == 0:
                        # one ap_gather covering all 8 dep groups, then a DVE
                        # repack to chunk-major for contiguous matmul rhs
                        zs_cm = cmpool.tile([128, max(caps[1:]) * P, KC],
                                            BF16, tag="zscm")
                        nc.gpsimd.ap_gather(
                            out_ap=zs_cm[:, 0:Cs, :],
                            in_ap=zin[:, 0:C, :],
                            idxs_ap=idxg[:, goff: goff + Cs // 16].bitcast(I16),
                            channels=128, num_elems=C, d=KC, num_idxs=Cs,
                        )
                        zs_all = gpool.tile([128, KC, max(caps[1:]) * P],
                                            BF16, tag="zs")
                        nc.vector.tensor_copy(
                            out=zs_all[:, :, 0:Cs],
                            in_=zs_cm[:, 0:Cs, :].rearrange("p n c -> p c n"),
                        )
                        zs = zs_all[:, :, 0:cap]
                    else:
                        zs = zs_all[:, :, p * cap:(p + 1) * cap]
                    if small_resident:
                        wdp = env_small["wdall"][:, :, p, :]
                    else:
                        wdp = gpool.tile([128, KC, D], BF16, tag="wdp")
                        nc.sync.dma_start(
                            out=wdp,
                            in_=wdep_p.ap().rearrange(
                                "d (c p) m -> d p c m", p=128)[p],
                        )
                    for mmr in range(KC):
                        pdc = {}
                        for (n0, w) in _chunks(cap, 512):
                            pdc[n0] = psd.tile([128, 512], F32, tag="pd",
                                               name=f"pd{mmr}_{n0}")
                        for kb in range(KC):
                            for (n0, w) in _chunks(cap, 512):
                                nc.tensor.matmul(
                                    pdc[n0][:, 0:w],
                                    wdp[:, kb, mmr * 128:(mmr + 1) * 128],
                                    zs[:, kb, n0:n0 + w],
                                    start=(kb == 0), stop=(kb == KC - 1),
                                )
                        for (n0, w) in _chunks(cap, 512):
                            nc.scalar.activation(
                                out=zcs[:, p * cap + n0: p * cap + n0 + w, mmr],
                                in_=pdc[n0][:, 0:w],
                                func=AFT.Identity,
                                bias=bdep[:, mmr * 8 + p: mmr * 8 + p + 1],
                            )

                gin = None
                zclv = None
                if not big:
                    # pre-gather step-ordered children and batch the whole
                    # input projection (incl. biases) into Gin [128, 16, 9m]
                    sw = mw
                    zclv = pgin.tile([128, KC, 9 * sw], BF16,
                                     tag="zclv", name="zclv")
                    nc.vector.memset(zclv, 0.0)
                    zclv_cm = cmpool.tile([128, K * sw, KC], BF16,
                                          tag="zclvcm")
                    nc.gpsimd.ap_gather(
                        out_ap=zclv_cm,
                        in_ap=zcs[:, 0:max(Cs, 128), :],
                        idxs_ap=idxs[:, soff: soff + K * mw // 16].bitcast(I16),
                        channels=128, num_elems=max(Cs, 128), d=KC,
                        num_idxs=K * sw,
                    )
                    nc.vector.tensor_copy(
                        out=zclv[:, :, 0:K * sw],
                        in_=zclv_cm.rearrange("p n c -> p c n"),
                    )
                    nc.vector.tensor_copy(
                        out=zclv[:, :, 8 * sw:8 * sw + m],
                        in_=xown[:, :, xoff:xoff + m],
                    )
                    gin = pgin.tile([128, 16, 9 * sw], BF16,
                                    tag="gin", name="gin")
                    for (n0, w) in _chunks(9 * sw, 512):
                        for mm in range(16):
                            pd = psd.tile([128, 512], F32, tag="pd")
                            for kb in range(KC):
                                nc.tensor.matmul(
                                    pd[:, 0:w],
                                    wih[:, kb, mm * 128:(mm + 1) * 128],
                                    zclv[:, kb, n0:n0 + w],
                                    start=(kb == 0), stop=(kb == KC - 1),
                                )
                            nc.scalar.activation(
                                out=gin[:, mm, n0:n0 + w], in_=pd[:, 0:w],
                                func=AFT.Identity, bias=bsum[:, mm:mm + 1],
                            )

            if li == 0:
                pz5_stack.close()

            # --- recurrence ---
            if is_root:
                zout = None
            else:
                ztag = "za" if li % 2 == 0 else "zb"
                zout = zpool.tile([128, max(m, 16), KC], BF16, tag=ztag)

            if big:
                with tc.tile_pool(name=f"pzct{li}", bufs=1) as pzct, \
                        tc.tile_pool(name=f"pact{li}", bufs=1) as pact, \
                        tc.tile_pool(name=f"psg{li}", bufs=2, space="PSUM") as psg:
                    NR = 3   # chunk-major ring slots (steps)
                    NCM = 3  # child-major ring slots
                    zct = pzct.tile([128, KC, NR * m], BF16, name="zct_ring")
                    zct_cm = pzct.tile([128, NCM * m, KC], BF16, name="zctcm")

                    def gather_step(t):
                        s = t % NCM
                        nc.gpsimd.ap_gather(
                            out_ap=zct_cm[:, s * m:(s + 1) * m, :],
                            in_ap=zcs[:, 0:max(Cs, 128), :],
                            idxs_ap=idxs[:, soff + t * mw // 16:
                                         soff + (t + 1) * mw // 16].bitcast(I16),
                            channels=128, num_elems=max(Cs, 128), d=KC,
                            num_idxs=m,
                        )

                    def repack_step(t):
                        s, sr = t % NCM, t % NR
                        nc.vector.tensor_copy(
                            out=zct[:, :, sr * m:(sr + 1) * m],
                            in_=zct_cm[:, s * m:(s + 1) * m, :].rearrange(
                                "p n c -> p c n"),
                        )

                    gather_step(0)
                    repack_step(0)
                    gather_step(1)
                    repack_step(1)
                    if stop_after == "gather":
                        for t in range(2, K):
                            gather_step(t)
                            repack_step(t)
                    for t in range(0 if stop_after not in ("dep", "gather")
                                   else K + 1, K + 1):
                        if t + 2 < K:
                            gather_step(t + 2)
                        if t + 1 < K and t >= 1:
                            repack_step(t + 1)
                        if t < K:
                            rhs_in = zct[:, :, (t % NR) * m:(t % NR + 1) * m]
                        else:
                            rhs_in = xown[:, :, xoff:xoff + m]
                        # full-width (512) matmuls, one 4-gate PSUM tile per
                        # gate type; every tile row is a full PSUM bank so ih
                        # and hh accumulation groups may interleave across
                        # gates. Order: Ti-ih, Tf-ih (h-independent prefix
                        # covers the h(t-1) latency), then hh + evac.
                        gate_tiles = [("i", 0), ("f", 4), ("o", 8), ("g", 12)]
                        if t == 0:
                            present = ["i", "o", "g"]
                        elif t == K:
                            present = ["i", "f", "g"]
                        else:
                            present = ["i", "f", "o", "g"]
                        act = pact.tile([128, 16, 512], BF16, tag="g16",
                                        name="act16")
                        pts = {}

                        def mk(gname):
                            pts[gname] = psg.tile([128, 4, 512], F32, tag="pgab",
                                                  name=f"pt_{gname}")

                        def ih_mms(gname, g0):
                            for j in range(g0, g0 + 4):
                                for kb in range(KC):
                                    nc.tensor.matmul(
                                        pts[gname][:, j - g0, :],
                                        wih[:, kb, j * 128:(j + 1) * 128],
                                        rhs_in[:, kb, 0:m],
                                        start=(kb == 0),
                                        stop=(t == 0 and kb == KC - 1),
                                    )

                        def hh_mms(gname, g0):
                            if t == 0:
                                return
                            for j in range(g0, g0 + 4):
                                for kb in range(KC):
                                    nc.tensor.matmul(
                                        pts[gname][:, j - g0, :],
                                        whh[:, kb, j * 128:(j + 1) * 128],
                                        h[:, kb, 0:m],
                                        start=False, stop=(kb == KC - 1),
                                    )

                        def evac(gname, g0):
                            for j in range(g0, g0 + 4):
                                nc.scalar.activation(
                                    out=act[:, j, :],
                                    in_=pts[gname][:, j - g0, :],
                                    func=(AFT.Tanh if j >= 12 else AFT.Sigmoid),
                                    bias=bsum[:, j:j + 1],
                                )

                        order = [g for g in ("i", "f", "o", "g") if g in present]
                        g0of = dict(gate_tiles)
                        # first two tiles: ih prefix, then their hh
                        mk(order[0]); ih_mms(order[0], g0of[order[0]])
                        mk(order[1]); ih_mms(order[1], g0of[order[1]])
                        hh_mms(order[0], g0of[order[0]])
                        evac(order[0], g0of[order[0]])
                        hh_mms(order[1], g0of[order[1]])
                        evac(order[1], g0of[order[1]])
                        for gname in order[2:]:
                            mk(gname)
                            ih_mms(gname, g0of[gname])
                            hh_mms(gname, g0of[gname])
                            evac(gname, g0of[gname])
                        csl = c[:, :, 0:m]
                        if t == 0:
                            nc.vector.tensor_mul(
                                csl, act[:, 0:4, :], act[:, 12:16, :])
                        else:
                            nc.vector.tensor_mul(csl, act[:, 4:8, :], csl)
                            tmp = npool.tile([128, KC, 512], F32, tag="nl")
                            nc.vector.tensor_mul(
                                tmp, act[:, 0:4, :], act[:, 12:16, :])
                            nc.vector.tensor_add(csl, csl, tmp)
                        if t < K:
                            tcv = npool.tile([128, KC, 512], F32, tag="nl")
                            nc.scalar.activation(
                                out=tcv, in_=csl, func=AFT.Tanh)
                            nc.vector.tensor_mul(
                                h[:, :, 0:m], act[:, 8:12, :], tcv)
                        else:
                            nc.vector.tensor_copy(
                                out=zout[:, 0:m, :].rearrange("p n c -> p c n"),
                                in_=csl)
            else:
                with tc.tile_pool(name=f"psg{li}", bufs=2, space="PSUM") as psg:
                    for t in range(K + 1):
                        mms = (list(range(16)) if t < K
                               else list(range(8)) + list(range(12, 16)))
                        if t > 0:
                            pg = psg.tile([128, 16, max(m, 16)], F32, tag="pgs")
                            # identity matmuls preload gin into PSUM (setting
                            # has_written), then W_hh accumulates on top; one
                            # identity mm per PSUM bank.
                            mpad = max(m, 16)
                            rows_per_bank = max(512 // mpad, 1)
                            for r0 in range(0, 16, rows_per_bank):
                                r1 = min(r0 + rows_per_bank, 16)
                                nc.tensor.matmul(
                                    pg[:, r0:r1, 0:m],
                                    ident,
                                    gin[:, r0:r1, t * mw:t * mw + m],
                                    start=True, stop=False,
                                )
                            for mm in mms:
                                for kb in range(KC):
                                    nc.tensor.matmul(
                                        pg[:, mm, 0:m],
                                        whh[:, kb, mm * 128:(mm + 1) * 128],
                                        h[:, kb, 0:m],
                                        start=False, stop=(kb == KC - 1),
                                    )
                            gsrc = pg
                        else:
                            gsrc = gin

                        nsig = 12 if t < K else 8
                        sig = npool.tile([128, 12, max(m, 16)], F32, tag="snl12")
                        nc.scalar.activation(
                            out=sig[:, 0:nsig, 0:m], in_=gsrc[:, 0:nsig, 0:m],
                            func=AFT.Sigmoid,
                        )
                        # keep-warm: punctuate the serial chain with PE work so
                        # the HAM clock gate stays at full rate
                        warm = psg.tile([128, 64], F32, tag="warm")
                        nc.tensor.matmul(
                            warm[0:m, 0:m], sig[:, 0, 0:m], sig[:, 0, 0:m],
                            start=True, stop=True,
                        )
                        si = sig[:, 0:4, 0:m]
                        fs = sig[:, 4:8, 0:m]
                        os_ = sig[:, 8:12, 0:m]
                        gt = npool.tile([128, KC, max(m, 16)], F32, tag="snl")
                        nc.scalar.activation(
                            out=gt[:, :, 0:m], in_=gsrc[:, 12:16, 0:m], func=AFT.Tanh
                        )
                        csl = c[:, :, 0:m]
                        if t == 0:
                            nc.vector.tensor_mul(csl, si, gt[:, :, 0:m])
                        else:
                            nc.vector.tensor_mul(csl, fs, csl)
                            tmp = npool.tile([128, KC, max(m, 16)], F32, tag="snl")
                            nc.vector.tensor_mul(
                                tmp[:, :, 0:m], si, gt[:, :, 0:m]
                            )
                            nc.vector.tensor_add(csl, csl, tmp[:, :, 0:m])
                        if t < K:
                            tcv = npool.tile([128, KC, max(m, 16)], F32, tag="snl")
                            nc.scalar.activation(
                                out=tcv[:, :, 0:m], in_=csl, func=AFT.Tanh
                            )
                            warm2 = psg.tile([128, 64], F32, tag="warm")
                            nc.tensor.matmul(
                                warm2[0:m, 0:m], tcv[:, 0, 0:m], tcv[:, 0, 0:m],
                                start=True, stop=True,
                            )
                            nc.vector.tensor_mul(
                                h[:, :, 0:m], os_, tcv[:, :, 0:m]
                            )
                        elif not is_root:
                            nc.vector.tensor_copy(
                                out=zout[:, 0:m, :].rearrange("p n c -> p c n"),
                                in_=csl)

            lvl_stack.close()
            if li == 0:
                zcs0_stack.close()
            if is_root:
                c_final = c
            zin = zout
            goff += giw[li]
            soff += siw[li]
            if stop_after in ("dep", "gather"):
                zin = zcs
                break

        if n_levels <= 0:
            zcs0_stack.close()
        if "pool" in env_small:
            env_small["pool"].close()
        if n_levels == len(LEVEL_SHAPES):
            nc.sync.dma_start(
                out=out_p.ap().rearrange("a (c p) -> p c a", p=128),
                in_=c_final[:, :, 0:1],
            )
        else:
            if n_levels <= 0:
                src_t = z5
            else:
                src_t = zin
            of32 = npool.tile([128, KC, 1], F32, tag="of32")
            if src_t is z5:
                nc.vector.tensor_copy(out=of32, in_=src_t[:, :, 0:1])
            elif stop_after in ("dep", "gather"):
                nc.vector.tensor_copy(
                    out=of32[:, :, 0], in_=src_t[:, 0, :])
            else:
                nc.vector.tensor_copy(
                    out=of32[:, :, 0], in_=src_t[:, 0, :])
            nc.gpsimd.dma_start(
                out=out_p.ap().rearrange("a (c p) -> p c a", p=128),
                in_=of32,
            )


GATE_PERM = np.concatenate([np.arange(0, 1024), np.arange(1536, 2048),
                            np.arange(1024, 1536)])  # [i, f, o, g]


def _prep_weights(W_ih, W_hh, W_dep, b_dep, b_ih, b_hh):
    wih_t_bf = np.ascontiguousarray(W_ih.T[:, GATE_PERM]).astype(BF)
    whh_t_bf = np.ascontiguousarray(W_hh.T[:, GATE_PERM]).astype(BF)
    wdep_t_bf = np.ascontiguousarray(W_dep.transpose(0, 2, 1)).astype(BF)
    bsum = (b_ih + b_hh).astype(np.float32)[GATE_PERM]
    bsum_t = np.ascontiguousarray(bsum.reshape(16, 128).T)
    bdep_t = np.ascontiguousarray(
        b_dep.T.reshape(KC, 128, P).transpose(1, 0, 2).reshape(128, KC * P))
    return wih_t_bf, whh_t_bf, wdep_t_bf, bsum_t, bdep_t


def _prep_core_inputs(core, embeddings, dep_types, wih_t_bf, whh_t_bf, wdep_t_bf,
                      bsum_t, bdep_t, caps):
    emb = embeddings

    cap0 = caps[0]
    leaf_emb = emb[OFF[5] + LEAF * core: OFF[5] + LEAF * (core + 1)]
    leaf_dep = dep_types[OFF[5] + LEAF * core: OFF[5] + LEAF * (core + 1)]
    cols = np.zeros(P * cap0, dtype=np.int64)
    for p in range(P):
        idx_p = np.where(leaf_dep == p)[0]
        cols[p * cap0: p * cap0 + len(idx_p)] = idx_p
    xleaf = np.ascontiguousarray(leaf_emb[cols].T).astype(BF)

    xown_cols = []
    for (_, C, m, _, is_root) in LEVEL_SHAPES:
        if is_root:
            xown_cols.append(emb[0:1])
        else:
            lvl = {512: 4, 64: 3, 8: 2, 1: 1}[m]
            s = OFF[lvl] + m * core
            xown_cols.append(emb[s:s + m])
    xown = np.ascontiguousarray(np.concatenate(xown_cols, axis=0).T).astype(BF)

    giw_cols = []
    siw_cols = []
    for li, (_, C, m, _, is_root) in enumerate(LEVEL_SHAPES):
        cap = caps[li]
        mw = _ceil32(m)
        if is_root:
            deps = dep_types[1:9]
        else:
            lvl = {512: 4, 64: 3, 8: 2, 1: 1}[m]
            s = OFF[lvl + 1] + C * core
            deps = dep_types[s:s + C]
        pos_of_child = np.zeros(C, dtype=np.int64)
        for p in range(P):
            idx_p = np.where(deps == p)[0]
            pos_of_child[idx_p] = p * cap + np.arange(len(idx_p))
            if li != 0:
                giw_cols.append(_wrap_idx(idx_p, cap // 16))
        for t in range(K):
            child = np.arange(m) * K + t
            siw_cols.append(_wrap_idx(pos_of_child[child], mw // 16))
    idxg = np.concatenate(giw_cols, axis=1)
    idxs = np.concatenate(siw_cols, axis=1)

    return {
        "xleaf": xleaf,
        "xown": xown,
        "ident": np.eye(128, dtype=np.float32).astype(BF),
        "wih_t": wih_t_bf,
        "whh_t": whh_t_bf,
        "wdep_t": wdep_t_bf,
        "bsum_t": bsum_t,
        "bdep_t": bdep_t,
        "idxg": idxg,
        "idxs": idxs,
    }


_CACHED = {}


def kernel(embeddings, dep_types, W_dep, b_dep, W_ih, W_hh, b_ih, b_hh):
    embeddings = np.asarray(embeddings, dtype=np.float32)
    dep_types = np.asarray(dep_types)
    W_dep = np.asarray(W_dep, dtype=np.float32)
    b_dep = np.asarray(b_dep, dtype=np.float32)
    W_ih = np.asarray(W_ih, dtype=np.float32)
    W_hh = np.asarray(W_hh, dtype=np.float32)
    b_ih = np.asarray(b_ih, dtype=np.float32)
    b_hh = np.asarray(b_hh, dtype=np.float32)

    # per-level dep-group capacities (max group size over cores, ceil to 16)
    caps = []
    for (_, C, m, _, is_root) in LEVEL_SHAPES:
        if is_root:
            mx = int(np.bincount(dep_types[1:9], minlength=P).max())
        else:
            lvl = {512: 4, 64: 3, 8: 2, 1: 1}[m]
            mx = 0
            for c in range(NCORES):
                s = OFF[lvl + 1] + C * c
                mx = max(mx, int(np.bincount(dep_types[s:s + C],
                                             minlength=P).max()))
        caps.append(max(_ceil32(mx), 32))

    wih_t_bf, whh_t_bf, wdep_t_bf, bsum_t, bdep_t = _prep_weights(
        W_ih, W_hh, W_dep, b_dep, b_ih, b_hh)

    key = tuple(caps)
    if key not in _CACHED:
        _CACHED[key] = build_program(caps)
    nc = _CACHED[key]

    in_maps = [
        _prep_core_inputs(c, embeddings, dep_types, wih_t_bf, whh_t_bf,
                          wdep_t_bf, bsum_t, bdep_t, caps)
        for c in range(NCORES)
    ]
    res = run_bass_kernel_spmd(nc, in_maps, list(range(NCORES)))
    out = np.asarray(res.results[0]["out"], dtype=np.float32).reshape(1, D)
    return out
